# revision 16
# baseline (speedup 1.0000x reference)
"""Multi-head attention (B=4, L=2048, D=512, H=8) on 8 Trainium2 cores.

Sharding: core c handles batch b = c//2, query rows [(c%2)*1024, +1024).

Key trick: the key-mask zeroes ~half the KV positions and is known on the
host, so K/V are COMPACTED on the host to the unmasked positions (padded
to a multiple of 128; pad columns get a -1e30 score bias so exp()=0).
This halves scores/exp/attnV work. Each core projects the compacted K/V
for its whole batch itself (cheap), so no collective is needed.

Schedule: the scalar engine (72 exps of [128,1024]) and the PE are
co-critical; the layout keeps both near-saturated:
 - warmup matmuls + a dummy exp run during the DMA preamble (HAM warm,
   ACT exp tables resident before the first real chunk)
 - only Q/K dmodel-chunk 0 is projected up front; V projection runs JIT
   inside head 0, remaining Q/K chunks stream into later heads' PE slack
 - attnV lags scores by 2 chunks so PE never waits on exp or the
   previous head's normalize
 - output-projection partials for the first q-tiles are issued before
   the final normalize; the output bias is folded into the matmul and
   the PSUM->SBUF moves run on the then-idle scalar engine

Device layouts (per core):
  xqT (512, 1024), xkT/xvT (512, KVCAP)  inputs, dmodel on partitions
  qT (128, 1024) x4 / kT (128, KVCAP) x4 projections kept transposed:
      head h lives in dmodel-chunk tile h//2 at partition offset 64*(h%2)
  V (128, 520) x KVC   V natural layout per kv chunk; head h at cols
      [65h, 65h+64), col 65h+64 = ones (softmax denominator)
  scores (128kv, 1024q) PSUM; exp+mask+scale fused into one ACT op
  xs (65, 1024) PSUM, row 64 = softmax denominator
"""
import numpy as np
import ml_dtypes

import concourse.bacc as bacc
import concourse.bass as bass
import concourse.mybir as mybir
import concourse.tile as tile
from concourse.bass_utils import run_bass_kernel_spmd

F32 = mybir.dt.float32
BF16 = mybir.dt.bfloat16
AF = mybir.ActivationFunctionType

B, L, D = 4, 2048, 512
H, DK = 8, 64
N_CORES = 8
LQ = L // 2            # query rows per core
P = 128
QT = LQ // P           # 8 query tiles of 128
MC = D // P            # 4 dmodel chunks
MASK_BIAS = np.float32(-1e30)

MM_DT = BF16
MM_NP = ml_dtypes.bfloat16

_cache = {}


def _plan(mask):
    """KV chunk count after host-side compaction (multiple-of-128 pad)."""
    counts = np.asarray(mask).astype(bool).sum(axis=1)
    kvc = int(np.ceil((counts.max() + 1e-9) / P))
    return max(kvc, 2)


def _build(kvc):
    kvcap = kvc * P
    nc = bacc.Bacc("TRN2", target_bir_lowering=False, debug=False,
                   num_devices=N_CORES)

    xqT_d = nc.dram_tensor("xqT", [D, LQ], MM_DT, kind="ExternalInput").ap()
    xkT_d = nc.dram_tensor("xkT", [D, kvcap], MM_DT, kind="ExternalInput").ap()
    xvT_d = nc.dram_tensor("xvT", [D, kvcap], MM_DT, kind="ExternalInput").ap()
    wq_d = nc.dram_tensor("wq", [D, D], MM_DT, kind="ExternalInput").ap()
    wk_d = nc.dram_tensor("wk", [D, D], MM_DT, kind="ExternalInput").ap()
    wv_d = nc.dram_tensor("wv", [D, D], MM_DT, kind="ExternalInput").ap()
    wo_d = nc.dram_tensor("wo", [D, D], MM_DT, kind="ExternalInput").ap()
    cst_d = nc.dram_tensor("cst", [P, 2 * MC + kvc], F32,
                           kind="ExternalInput").ap()
    bv_d = nc.dram_tensor("bv", [1, D], F32, kind="ExternalInput").ap()
    bob_d = nc.dram_tensor("bob", [1, D], MM_DT, kind="ExternalInput").ap()
    out_d = nc.dram_tensor("out", [LQ, D], MM_DT, kind="ExternalOutput").ap()

    # 512-column blocks of the compacted KV extent
    kblk = []
    off = 0
    while off < kvcap:
        blk = min(512, kvcap - off)
        kblk.append((off, blk))
        off += blk
    NB = len(kblk)

    with tile.TileContext(nc) as tc:
        with tc.tile_pool(name="const", bufs=1) as cpool, \
             tc.tile_pool(name="xin", bufs=1) as xpool, \
             tc.tile_pool(name="proj", bufs=1) as prpool, \
             tc.tile_pool(name="attn", bufs=6) as apool, \
             tc.tile_pool(name="norm", bufs=2) as npool, \
             tc.tile_pool(name="outp", bufs=2) as opool, \
             tc.tile_pool(name="ps", bufs=3, space="PSUM") as ps:

            def alloc_chunks(pool, ap2d, nm):
                return [pool.tile([P, ap2d.shape[1]], ap2d.dtype,
                                  tag=f"{nm}{kc}", name=f"{nm}{kc}")
                        for kc in range(MC)]

            def load_all(eng, tiles, ap2d):
                for kc in range(MC):
                    eng.dma_start(tiles[kc][:], ap2d[kc * P:(kc + 1) * P, :])

            def load_block(eng, tiles, ap2d, off, blk):
                for kc in range(MC):
                    eng.dma_start(
                        tiles[kc][:, off:off + blk],
                        ap2d[kc * P:(kc + 1) * P, off:off + blk])

            # ---- DMA, split across the sync and scalar (ACT) queues so
            # descriptor issue (~0.6us/op) isn't serialized ----
            wq = alloc_chunks(cpool, wq_d, "wq")
            xqT = alloc_chunks(xpool, xqT_d, "xq")
            wk = alloc_chunks(cpool, wk_d, "wk")
            xkT = alloc_chunks(xpool, xkT_d, "xk")
            wv = alloc_chunks(cpool, wv_d, "wv")
            xvT = alloc_chunks(xpool, xvT_d, "xv")
            wo = alloc_chunks(cpool, wo_d, "wo")
            load_all(nc.scalar, wk, wk_d)
            load_all(nc.scalar, wv, wv_d)
            load_all(nc.sync, wq, wq_d)
            load_all(nc.sync, xqT, xqT_d)
            cst = cpool.tile([P, 2 * MC + kvc], F32, tag="cst", name="cst")
            nc.sync.dma_start(cst[:], cst_d)
            bq = cst[:, 0:MC]
            bk = cst[:, MC:2 * MC]
            mb = cst[:, 2 * MC:]
            load_block(nc.sync, xkT, xkT_d, *kblk[0])
            load_block(nc.sync, xvT, xvT_d, *kblk[0])
            bv = cpool.tile([1, D], F32, tag="bv", name="bv")
            nc.sync.dma_start(bv[:], bv_d)
            bob = cpool.tile([1, D], MM_DT, tag="bob", name="bob")
            for off, blk in kblk[1:]:
                load_block(nc.sync, xkT, xkT_d, off, blk)
                load_block(nc.sync, xvT, xvT_d, off, blk)
            load_all(nc.sync, wo, wo_d)
            nc.sync.dma_start(bob[:], bob_d)

            ones_w = cpool.tile([1, D], MM_DT)
            nc.vector.memset(ones_w[:], 1.0)
            bv_bc = cpool.tile([P, D], F32)
            nc.gpsimd.partition_broadcast(bv_bc[:], bv[:])

            # ---- PE warmup (HAM un-throttle) ----
            wps = ps.tile([P, LQ], F32, tag="sc", name="wps")
            for i in range(10):
                nc.tensor.matmul(wps[:, 0:512], ones_w[0:1, 0:P],
                                 ones_w[0:1, :], start=True, stop=True)

            # ---- persistent SBUF tiles ----
            qT = [prpool.tile([P, LQ], MM_DT, tag=f"qT{m}", name=f"qT{m}")
                  for m in range(MC)]
            kT = [prpool.tile([P, kvcap], MM_DT, tag=f"kT{m}", name=f"kT{m}")
                  for m in range(MC)]
            V = [prpool.tile([P, H * 65], MM_DT, tag=f"V{t}", name=f"V{t}")
                 for t in range(kvc)]
            xsT2 = [prpool.tile([P, LQ], MM_DT, tag=f"xs{hp}", name=f"xsT2_{hp}")
                    for hp in range(MC)]
            xsU = [prpool.tile([64, LQ], MM_DT, tag=f"xsU{h}", name=f"xsU{h}")
                   for h in range(H - 1)]

            def q_unit(m, engine):
                pp = ps.tile([P, LQ], F32, tag="sc", name=f"qpp{m}")
                for kc in range(MC):
                    for s in range(2):
                        nc.tensor.matmul(
                            pp[:, s * 512:(s + 1) * 512],
                            wq[kc][:, m * P:(m + 1) * P],
                            xqT[kc][:, s * 512:(s + 1) * 512],
                            start=kc == 0, stop=kc == MC - 1)
                if engine == "act":
                    nc.scalar.activation(qT[m][:], pp[:], AF.Identity,
                                         bias=bq[:, m:m + 1])
                else:
                    nc.vector.tensor_scalar_add(qT[m][:], pp[:], bq[:, m:m + 1])

            def k_unit(m, bi):
                off, blk = kblk[bi]
                pk = ps.tile([P, LQ], F32, tag="sc", name=f"pk{m}_{bi}")
                for kc in range(MC):
                    nc.tensor.matmul(
                        pk[:, 0:blk],
                        wk[kc][:, m * P:(m + 1) * P],
                        xkT[kc][:, off:off + blk],
                        start=kc == 0, stop=kc == MC - 1)
                nc.vector.tensor_scalar_add(kT[m][:, off:off + blk],
                                            pk[:, 0:blk], bk[:, m:m + 1])

            def v_unit(t):
                pv = ps.tile([P, LQ], F32, tag="sc", name=f"pv{t}")
                for kc in range(MC):
                    nc.tensor.matmul(pv[:, 0:D],
                                     xvT[kc][:, t * P:(t + 1) * P],
                                     wv[kc][:, :],
                                     start=kc == 0, stop=kc == MC - 1)
                vv = V[t].rearrange("p (g d) -> p g d", d=65)
                nc.vector.tensor_add(
                    vv[:, :, 0:64],
                    pv[:, 0:D].rearrange("p (g d) -> p g d", d=64),
                    bv_bc.rearrange("p (g d) -> p g d", d=64))
                nc.vector.memset(vv[:, :, 64:65], 1.0)

            q_unit(0, "act")
            k_unit(0, 0)

            # ---- flash attention ----
            def scores_mms(h, c, qh, ss):
                hp, po = h // 2, 64 * (h % 2)
                nc.tensor.matmul(
                    ss[:, qh * 512:(qh + 1) * 512],
                    kT[hp][po:po + 64, c * P:(c + 1) * P],
                    qT[hp][po:po + 64, qh * 512:(qh + 1) * 512],
                    start=True, stop=True)

            def exp_chunk(h, c, ss):
                a = apool.tile([P, LQ], MM_DT, tag="at", name=f"at_h{h}_{c}")
                nc.scalar.activation(a[:], ss[:], AF.Exp,
                                     bias=mb[:, c:c + 1], scale=0.125)
                return a

            def attnv_chunk(h, c, xs, a):
                for qh in range(2):
                    nc.tensor.matmul(
                        xs[:, qh * 512:(qh + 1) * 512],
                        V[c][:, 65 * h:65 * h + 65],
                        a[:, qh * 512:(qh + 1) * 512],
                        start=c == 0, stop=c == kvc - 1)

            def normalize(h, xs):
                hp, po = h // 2, 64 * (h % 2)
                if h < H - 1:
                    rec = npool.tile([1, LQ], F32, tag="rec")
                    nc.vector.reciprocal_approx_fast(rec[:], xs[64:65, :])
                    # free the PSUM accumulator with one DVE copy; the
                    # broadcast+mul then run off the critical path
                    nc.vector.tensor_copy(xsU[h][:], xs[0:64, :])
                    bc = npool.tile([64, LQ], F32, tag="bc")
                    nc.gpsimd.partition_broadcast(bc[:], rec[:])
                    nc.vector.tensor_mul(xsT2[hp][po:po + 64, :],
                                         xsU[h][:], bc[:])
                else:
                    # last head: per-qh chains so the output projection of
                    # the first query half can start earlier
                    for qh in range(2):
                        sl = slice(qh * 512, (qh + 1) * 512)
                        rec = npool.tile([1, 512], F32, tag="rec")
                        nc.vector.reciprocal_approx_fast(rec[:], xs[64:65, sl])
                        bc = npool.tile([64, 512], F32, tag="bc")
                        nc.gpsimd.partition_broadcast(bc[:], rec[:])
                        nc.vector.tensor_mul(xsT2[hp][po:po + 64, sl],
                                             xs[0:64, sl], bc[:])

            # ---- output projection helpers ----
            otile = {}

            def o_mms(q2, hps):
                if q2 not in otile:
                    otile[q2] = ps.tile([P, LQ], F32, tag="sc", name=f"po{q2}")
                po_ = otile[q2]
                for sub in range(2):
                    qt = 2 * q2 + sub
                    for hp in hps:
                        nc.tensor.matmul(po_[:, sub * 512:(sub + 1) * 512],
                                         xsT2[hp][:, qt * P:(qt + 1) * P],
                                         wo[hp][:, :],
                                         start=hp == 0, stop=False)

            def o_finish(q2):
                po_ = otile[q2]
                for sub in range(2):
                    nc.tensor.matmul(po_[:, sub * 512:(sub + 1) * 512],
                                     ones_w[0:1, 0:P], bob[0:1, :],
                                     start=False, stop=True)
                for sub in range(2):
                    qt = 2 * q2 + sub
                    osb = opool.tile([P, D], MM_DT, tag="osb", bufs=4,
                                     name=f"osb{qt}")
                    if sub == 0:
                        nc.scalar.copy(osb[:], po_[:, 0:512])
                        nc.sync.dma_start(out_d[qt * P:(qt + 1) * P, :],
                                          osb[:])
                    else:
                        nc.vector.tensor_copy(osb[:], po_[:, 512:1024])
                        nc.scalar.dma_start(out_d[qt * P:(qt + 1) * P, :],
                                            osb[:])

            # side work: head 0 carries the V JIT (one unit per chunk) and
            # the remaining K blocks of m=0; dmodel chunk m is projected
            # during heads 2m-2 / 2m-1.
            side = {h: [[] for _ in range(kvc)] for h in range(H)}
            for t in range(kvc):
                side[0][t].append(lambda t=t: v_unit(t))
            for j, bi in enumerate(range(1, NB)):
                step = min(2 + 3 * j, 4 * bi - 2, kvc - 1)
                side[0][step].append(lambda bi=bi: k_unit(0, bi))
            for m in range(1, MC):
                units = [lambda m=m: q_unit(m, "dve")]
                units += [lambda m=m, bi=bi: k_unit(m, bi) for bi in range(NB)]
                ha, hb = max(2 * m - 2, 1), 2 * m - 1
                for i, u in enumerate(units):
                    h = ha if (i < 2 and ha != hb) else hb
                    side[h][min(1 + i, kvc - 1)].append(u)

            # per head: chunk-pipelined scores/exp with attnV lagging two
            # chunks; each head's epilogue (attnV drain + normalize) is
            # emitted after the NEXT head's first chunk so the scalar
            # engine never starves at head boundaries
            def make_epilogue(h, xs, ats, done_box):
                def run():
                    d = done_box[0]
                    while d < kvc:
                        attnv_chunk(h, d, xs, ats.pop(d))
                        d += 1
                    done_box[0] = d
                    if h == H - 1:
                        for q2 in (0, 1, 2):
                            o_mms(q2, range(MC - 1))
                    normalize(h, xs)
                return run

            pending = None
            for h in range(H):
                xs = ps.tile([65, LQ], F32, tag="xs", bufs=1, name=f"xs_h{h}")
                ats = {}
                done_box = [0]
                for c in range(kvc):
                    ss = ps.tile([P, LQ], F32, tag="sc", name=f"ss_h{h}_{c}")
                    for qh in range(2):
                        scores_mms(h, c, qh, ss)
                    ats[c] = exp_chunk(h, c, ss)
                    if c == 0 and pending is not None:
                        pending()
                        pending = None
                    for u in side[h][c]:
                        u()
                    if c >= 2:
                        attnv_chunk(h, c - 2, xs, ats.pop(c - 2))
                        done_box[0] = c - 1
                pending = make_epilogue(h, xs, ats, done_box)
            pending()

            # ---- output projection tail ----
            for q2 in (0, 1, 2):
                o_mms(q2, [MC - 1])
                o_finish(q2)
            o_mms(3, range(MC))
            o_finish(3)

    nc.compile()
    return nc


def _host_inputs(query, key, value, mask, Wq, bq, Wk, bk, Wv, bv, Wo, bo,
                 kvc=None):
    """Build the 8 per-core input maps (all rank-dependence lives here)."""
    f32 = np.float32
    if kvc is None:
        kvc = _plan(mask)
    kvcap = kvc * P
    wq_ = np.ascontiguousarray(Wq).astype(MM_NP)
    wk_ = np.ascontiguousarray(Wk).astype(MM_NP)
    wv_ = np.ascontiguousarray(Wv).astype(MM_NP)
    wo_ = np.ascontiguousarray(Wo).astype(MM_NP)
    bq_ = bq.astype(f32).reshape(MC, P).T
    bk_ = bk.astype(f32).reshape(MC, P).T
    bv_ = bv.astype(f32).reshape(1, D)
    bob_ = bo.astype(MM_NP).reshape(1, D)
    in_maps = []
    per_batch = {}
    for b in range(B):
        idx = np.flatnonzero(np.asarray(mask[b]) != 0)
        n = len(idx)
        xk = np.zeros((kvcap, D), f32)
        xv = np.zeros((kvcap, D), f32)
        xk[:n] = np.asarray(key[b], f32)[idx]
        xv[:n] = np.asarray(value[b], f32)[idx]
        mbias = np.full(kvcap, MASK_BIAS, f32)
        mbias[:n] = 0.0
        cst = np.concatenate([bq_, bk_, mbias.reshape(kvc, P).T], axis=1)
        per_batch[b] = (
            np.ascontiguousarray(xk.T).astype(MM_NP),
            np.ascontiguousarray(xv.T).astype(MM_NP),
            np.ascontiguousarray(cst),
        )
    for c in range(N_CORES):
        b, half = c // 2, c % 2
        sl = slice(half * LQ, (half + 1) * LQ)
        xqT = np.ascontiguousarray(np.asarray(query[b], f32)[sl].T).astype(MM_NP)
        xkT_, xvT_, cst_ = per_batch[b]
        in_maps.append({
            "xqT": xqT, "xkT": xkT_, "xvT": xvT_,
            "wq": wq_, "wk": wk_, "wv": wv_, "wo": wo_,
            "cst": cst_, "bv": bv_, "bob": bob_,
        })
    return in_maps


def kernel(query, key, value, mask, Wq, bq, Wk, bk, Wv, bv, Wo, bo):
    kvc = _plan(mask)
    if kvc not in _cache:
        _cache[kvc] = _build(kvc)
    _cache["nc"] = _cache[kvc]
    nc = _cache[kvc]
    in_maps = _host_inputs(query, key, value, mask,
                           Wq, bq, Wk, bk, Wv, bv, Wo, bo, kvc=kvc)
    res = run_bass_kernel_spmd(nc, in_maps, list(range(N_CORES))).results
    out = np.empty((B, L, D), np.float32)
    for c in range(N_CORES):
        b, half = c // 2, c % 2
        out[b, half * LQ:(half + 1) * LQ, :] = res[c]["out"].astype(np.float32)
    return out


# revision 17
# speedup vs baseline: 1.0449x; 1.0449x over previous
"""Multi-head attention (B=4, L=2048, D=512, H=8) on 8 Trainium2 cores.

Sharding: core c handles batch b = c//2, query rows [(c%2)*1024, +1024).

Key trick: the key-mask zeroes ~half the KV positions and is known on the
host, so K/V are COMPACTED on the host to the unmasked positions (padded
to a multiple of 128; pad columns get a -1e30 score bias so exp()=0).
This halves scores/exp/attnV work. Each core projects the compacted K/V
for its whole batch itself (cheap), so no collective is needed.

Schedule: the scalar engine (72 exps of [128,1024]) and the PE are
co-critical; the layout keeps both near-saturated:
 - warmup matmuls + a dummy exp run during the DMA preamble (HAM warm,
   ACT exp tables resident before the first real chunk)
 - only Q/K dmodel-chunk 0 is projected up front; V projection runs JIT
   inside head 0, remaining Q/K chunks stream into later heads' PE slack
 - attnV lags scores by 2 chunks so PE never waits on exp or the
   previous head's normalize
 - output-projection partials for the first q-tiles are issued before
   the final normalize; the output bias is folded into the matmul and
   the PSUM->SBUF moves run on the then-idle scalar engine

Device layouts (per core):
  xqT (512, 1024), xkT/xvT (512, KVCAP)  inputs, dmodel on partitions
  qT (128, 1024) x4 / kT (128, KVCAP) x4 projections kept transposed:
      head h lives in dmodel-chunk tile h//2 at partition offset 64*(h%2)
  V (128, 520) x KVC   V natural layout per kv chunk; head h at cols
      [65h, 65h+64), col 65h+64 = ones (softmax denominator)
  scores (128kv, 1024q) PSUM; exp+mask+scale fused into one ACT op
  xs (65, 1024) PSUM, row 64 = softmax denominator
"""
import numpy as np
import ml_dtypes

import concourse.bacc as bacc
import concourse.bass as bass
import concourse.mybir as mybir
import concourse.tile as tile
from concourse.bass_utils import run_bass_kernel_spmd

F32 = mybir.dt.float32
BF16 = mybir.dt.bfloat16
AF = mybir.ActivationFunctionType

B, L, D = 4, 2048, 512
H, DK = 8, 64
N_CORES = 8
LQ = L // 2            # query rows per core
P = 128
QT = LQ // P           # 8 query tiles of 128
MC = D // P            # 4 dmodel chunks
MASK_BIAS = np.float32(-1e30)

MM_DT = BF16
MM_NP = ml_dtypes.bfloat16

_cache = {}


def _plan(mask):
    """KV chunk count after host-side compaction (multiple-of-128 pad)."""
    counts = np.asarray(mask).astype(bool).sum(axis=1)
    kvc = int(np.ceil((counts.max() + 1e-9) / P))
    return max(kvc, 2)


def _build(kvc):
    kvcap = kvc * P
    nc = bacc.Bacc("TRN2", target_bir_lowering=False, debug=False,
                   num_devices=N_CORES)

    xqT_d = nc.dram_tensor("xqT", [D, LQ], MM_DT, kind="ExternalInput").ap()
    xkT_d = nc.dram_tensor("xkT", [D, kvcap], MM_DT, kind="ExternalInput").ap()
    xvT_d = nc.dram_tensor("xvT", [D, kvcap], MM_DT, kind="ExternalInput").ap()
    wq_d = nc.dram_tensor("wq", [D, D], MM_DT, kind="ExternalInput").ap()
    wk_d = nc.dram_tensor("wk", [D, D], MM_DT, kind="ExternalInput").ap()
    wv_d = nc.dram_tensor("wv", [D, D], MM_DT, kind="ExternalInput").ap()
    wo_d = nc.dram_tensor("wo", [D, D], MM_DT, kind="ExternalInput").ap()
    cst_d = nc.dram_tensor("cst", [P, 2 * MC + kvc], F32,
                           kind="ExternalInput").ap()
    bv_d = nc.dram_tensor("bv", [1, D], F32, kind="ExternalInput").ap()
    bob_d = nc.dram_tensor("bob", [1, D], MM_DT, kind="ExternalInput").ap()
    out_d = nc.dram_tensor("out", [LQ, D], MM_DT, kind="ExternalOutput").ap()

    # 512-column blocks of the compacted KV extent
    kblk = []
    off = 0
    while off < kvcap:
        blk = min(512, kvcap - off)
        kblk.append((off, blk))
        off += blk
    NB = len(kblk)

    with tile.TileContext(nc) as tc:
        with tc.tile_pool(name="const", bufs=1) as cpool, \
             tc.tile_pool(name="xin", bufs=1) as xpool, \
             tc.tile_pool(name="proj", bufs=1) as prpool, \
             tc.tile_pool(name="attn", bufs=6) as apool, \
             tc.tile_pool(name="norm", bufs=2) as npool, \
             tc.tile_pool(name="outp", bufs=2) as opool, \
             tc.tile_pool(name="ps", bufs=3, space="PSUM") as ps:

            def alloc_chunks(pool, ap2d, nm):
                return [pool.tile([P, ap2d.shape[1]], ap2d.dtype,
                                  tag=f"{nm}{kc}", name=f"{nm}{kc}")
                        for kc in range(MC)]

            def load_all(eng, tiles, ap2d):
                for kc in range(MC):
                    eng.dma_start(tiles[kc][:], ap2d[kc * P:(kc + 1) * P, :])

            def load_block(eng, tiles, ap2d, off, blk):
                for kc in range(MC):
                    eng.dma_start(
                        tiles[kc][:, off:off + blk],
                        ap2d[kc * P:(kc + 1) * P, off:off + blk])

            # ---- DMA, split across the sync and scalar (ACT) queues so
            # descriptor issue (~0.6us/op) isn't serialized ----
            wq = alloc_chunks(cpool, wq_d, "wq")
            xqT = alloc_chunks(xpool, xqT_d, "xq")
            wk = alloc_chunks(cpool, wk_d, "wk")
            xkT = alloc_chunks(xpool, xkT_d, "xk")
            wv = alloc_chunks(cpool, wv_d, "wv")
            xvT = alloc_chunks(xpool, xvT_d, "xv")
            wo = alloc_chunks(cpool, wo_d, "wo")
            load_all(nc.scalar, wq, wq_d)
            load_all(nc.scalar, xqT, xqT_d)
            cst = cpool.tile([P, 2 * MC + kvc], F32, tag="cst", name="cst")
            nc.sync.dma_start(cst[:], cst_d)
            bq = cst[:, 0:MC]
            bk = cst[:, MC:2 * MC]
            mb = cst[:, 2 * MC:]
            load_all(nc.sync, wk, wk_d)
            load_block(nc.sync, xkT, xkT_d, *kblk[0])
            load_all(nc.sync, wv, wv_d)
            load_block(nc.sync, xvT, xvT_d, *kblk[0])
            bv = cpool.tile([1, D], F32, tag="bv", name="bv")
            nc.sync.dma_start(bv[:], bv_d)
            bob = cpool.tile([1, D], MM_DT, tag="bob", name="bob")
            for off, blk in kblk[1:]:
                load_block(nc.sync, xkT, xkT_d, off, blk)
                load_block(nc.sync, xvT, xvT_d, off, blk)
            load_all(nc.sync, wo, wo_d)
            nc.sync.dma_start(bob[:], bob_d)

            ones_w = cpool.tile([1, D], MM_DT)
            nc.vector.memset(ones_w[:], 1.0)
            bv_bc = cpool.tile([P, D], F32)
            nc.gpsimd.partition_broadcast(bv_bc[:], bv[:])

            # ---- PE warmup (HAM un-throttle) ----
            wps = ps.tile([P, LQ], F32, tag="sc", name="wps")
            for i in range(10):
                nc.tensor.matmul(wps[:, 0:512], ones_w[0:1, 0:P],
                                 ones_w[0:1, :], start=True, stop=True)

            # ---- persistent SBUF tiles ----
            qT = [prpool.tile([P, LQ], MM_DT, tag=f"qT{m}", name=f"qT{m}")
                  for m in range(MC)]
            kT = [prpool.tile([P, kvcap], MM_DT, tag=f"kT{m}", name=f"kT{m}")
                  for m in range(MC)]
            V = [prpool.tile([P, H * 65], MM_DT, tag=f"V{t}", name=f"V{t}")
                 for t in range(kvc)]
            xsT2 = [prpool.tile([P, LQ], MM_DT, tag=f"xs{hp}", name=f"xsT2_{hp}")
                    for hp in range(MC)]
            xsU = [prpool.tile([64, LQ], MM_DT, tag=f"xsU{h}", name=f"xsU{h}")
                   for h in range(H - 1)]

            def q_unit(m, engine):
                pp = ps.tile([P, LQ], F32, tag="sc", name=f"qpp{m}")
                for kc in range(MC):
                    for s in range(2):
                        nc.tensor.matmul(
                            pp[:, s * 512:(s + 1) * 512],
                            wq[kc][:, m * P:(m + 1) * P],
                            xqT[kc][:, s * 512:(s + 1) * 512],
                            start=kc == 0, stop=kc == MC - 1)
                if engine == "act":
                    nc.scalar.activation(qT[m][:], pp[:], AF.Identity,
                                         bias=bq[:, m:m + 1])
                else:
                    nc.vector.tensor_scalar_add(qT[m][:], pp[:], bq[:, m:m + 1])

            def k_unit(m, bi):
                off, blk = kblk[bi]
                pk = ps.tile([P, LQ], F32, tag="sc", name=f"pk{m}_{bi}")
                for kc in range(MC):
                    nc.tensor.matmul(
                        pk[:, 0:blk],
                        wk[kc][:, m * P:(m + 1) * P],
                        xkT[kc][:, off:off + blk],
                        start=kc == 0, stop=kc == MC - 1)
                nc.vector.tensor_scalar_add(kT[m][:, off:off + blk],
                                            pk[:, 0:blk], bk[:, m:m + 1])

            def v_unit(t):
                pv = ps.tile([P, LQ], F32, tag="sc", name=f"pv{t}")
                for kc in range(MC):
                    nc.tensor.matmul(pv[:, 0:D],
                                     xvT[kc][:, t * P:(t + 1) * P],
                                     wv[kc][:, :],
                                     start=kc == 0, stop=kc == MC - 1)
                vv = V[t].rearrange("p (g d) -> p g d", d=65)
                nc.vector.tensor_add(
                    vv[:, :, 0:64],
                    pv[:, 0:D].rearrange("p (g d) -> p g d", d=64),
                    bv_bc.rearrange("p (g d) -> p g d", d=64))
                nc.vector.memset(vv[:, :, 64:65], 1.0)

            q_unit(0, "act")
            k_unit(0, 0)

            # ---- flash attention ----
            def scores_mms(h, c, qh, ss):
                hp, po = h // 2, 64 * (h % 2)
                nc.tensor.matmul(
                    ss[:, qh * 512:(qh + 1) * 512],
                    kT[hp][po:po + 64, c * P:(c + 1) * P],
                    qT[hp][po:po + 64, qh * 512:(qh + 1) * 512],
                    start=True, stop=True)

            def exp_chunk(h, c, ss):
                a = apool.tile([P, LQ], MM_DT, tag="at", name=f"at_h{h}_{c}")
                nc.scalar.activation(a[:], ss[:], AF.Exp,
                                     bias=mb[:, c:c + 1], scale=0.125)
                return a

            def attnv_chunk(h, c, xs, a):
                for qh in range(2):
                    nc.tensor.matmul(
                        xs[:, qh * 512:(qh + 1) * 512],
                        V[c][:, 65 * h:65 * h + 65],
                        a[:, qh * 512:(qh + 1) * 512],
                        start=c == 0, stop=c == kvc - 1)

            def normalize(h, xs):
                hp, po = h // 2, 64 * (h % 2)
                if h < H - 1:
                    rec = npool.tile([1, LQ], F32, tag="rec")
                    nc.vector.reciprocal_approx_fast(rec[:], xs[64:65, :])
                    # free the PSUM accumulator with one DVE copy; the
                    # broadcast+mul then run off the critical path
                    nc.vector.tensor_copy(xsU[h][:], xs[0:64, :])
                    bc = npool.tile([64, LQ], F32, tag="bc")
                    nc.gpsimd.partition_broadcast(bc[:], rec[:])
                    nc.vector.tensor_mul(xsT2[hp][po:po + 64, :],
                                         xsU[h][:], bc[:])
                else:
                    # last head: per-qh chains so the output projection of
                    # the first query half can start earlier
                    for qh in range(2):
                        sl = slice(qh * 512, (qh + 1) * 512)
                        rec = npool.tile([1, 512], F32, tag="rec")
                        nc.vector.reciprocal_approx_fast(rec[:], xs[64:65, sl])
                        bc = npool.tile([64, 512], F32, tag="bc")
                        nc.gpsimd.partition_broadcast(bc[:], rec[:])
                        nc.vector.tensor_mul(xsT2[hp][po:po + 64, sl],
                                             xs[0:64, sl], bc[:])

            # ---- output projection helpers ----
            otile = {}

            def o_mms(q2, hps):
                if q2 not in otile:
                    otile[q2] = ps.tile([P, LQ], F32, tag="sc", name=f"po{q2}")
                po_ = otile[q2]
                for sub in range(2):
                    qt = 2 * q2 + sub
                    for hp in hps:
                        nc.tensor.matmul(po_[:, sub * 512:(sub + 1) * 512],
                                         xsT2[hp][:, qt * P:(qt + 1) * P],
                                         wo[hp][:, :],
                                         start=hp == 0, stop=False)

            def o_finish(q2):
                po_ = otile[q2]
                for sub in range(2):
                    nc.tensor.matmul(po_[:, sub * 512:(sub + 1) * 512],
                                     ones_w[0:1, 0:P], bob[0:1, :],
                                     start=False, stop=True)
                for sub in range(2):
                    qt = 2 * q2 + sub
                    osb = opool.tile([P, D], MM_DT, tag="osb", bufs=4,
                                     name=f"osb{qt}")
                    if sub == 0:
                        nc.scalar.copy(osb[:], po_[:, 0:512])
                        nc.sync.dma_start(out_d[qt * P:(qt + 1) * P, :],
                                          osb[:])
                    else:
                        nc.vector.tensor_copy(osb[:], po_[:, 512:1024])
                        nc.scalar.dma_start(out_d[qt * P:(qt + 1) * P, :],
                                            osb[:])

            # side work: head 0 carries the V JIT (one unit per chunk) and
            # the remaining K blocks of m=0; dmodel chunk m is projected
            # during heads 2m-2 / 2m-1.
            side = {h: [[] for _ in range(kvc)] for h in range(H)}
            for t in range(kvc):
                side[0][t].append(lambda t=t: v_unit(t))
            for j, bi in enumerate(range(1, NB)):
                step = min(2 + 3 * j, 4 * bi - 2, kvc - 1)
                side[0][step].append(lambda bi=bi: k_unit(0, bi))
            for m in range(1, MC):
                units = [lambda m=m: q_unit(m, "dve")]
                units += [lambda m=m, bi=bi: k_unit(m, bi) for bi in range(NB)]
                ha, hb = max(2 * m - 2, 1), 2 * m - 1
                slots = ([(ha, 3), (ha, 6), (hb, 1), (hb, 4), (hb, 7)]
                         if ha != hb else
                         [(hb, 1), (hb, 3), (hb, 5), (hb, 7), (hb, 8)])
                for (h, st), u in zip(slots, units):
                    side[h][min(st, kvc - 1)].append(u)

            # per head: chunk-pipelined scores/exp with attnV lagging two
            # chunks; each head's epilogue (attnV drain + normalize) is
            # emitted after the NEXT head's first chunk so the scalar
            # engine never starves at head boundaries
            def make_epilogue(h, xs, ats, done_box):
                def run():
                    d = done_box[0]
                    while d < kvc:
                        attnv_chunk(h, d, xs, ats.pop(d))
                        d += 1
                    done_box[0] = d
                    if h == H - 1:
                        for q2 in (0, 1, 2):
                            o_mms(q2, range(MC - 1))
                    normalize(h, xs)
                return run

            pending = None
            for h in range(H):
                xs = ps.tile([65, LQ], F32, tag="xs", bufs=1, name=f"xs_h{h}")
                ats = {}
                done_box = [0]
                for c in range(kvc):
                    ss = ps.tile([P, LQ], F32, tag="sc", name=f"ss_h{h}_{c}")
                    for qh in range(2):
                        scores_mms(h, c, qh, ss)
                    ats[c] = exp_chunk(h, c, ss)
                    if c == 0 and pending is not None:
                        pending()
                        pending = None
                    for u in side[h][c]:
                        u()
                    if c >= 2:
                        attnv_chunk(h, c - 2, xs, ats.pop(c - 2))
                        done_box[0] = c - 1
                pending = make_epilogue(h, xs, ats, done_box)
            pending()

            # ---- output projection tail ----
            for q2 in (0, 1, 2):
                o_mms(q2, [MC - 1])
                o_finish(q2)
            o_mms(3, range(MC))
            o_finish(3)

    nc.compile()
    return nc


def _host_inputs(query, key, value, mask, Wq, bq, Wk, bk, Wv, bv, Wo, bo,
                 kvc=None):
    """Build the 8 per-core input maps (all rank-dependence lives here)."""
    f32 = np.float32
    if kvc is None:
        kvc = _plan(mask)
    kvcap = kvc * P
    wq_ = np.ascontiguousarray(Wq).astype(MM_NP)
    wk_ = np.ascontiguousarray(Wk).astype(MM_NP)
    wv_ = np.ascontiguousarray(Wv).astype(MM_NP)
    wo_ = np.ascontiguousarray(Wo).astype(MM_NP)
    bq_ = bq.astype(f32).reshape(MC, P).T
    bk_ = bk.astype(f32).reshape(MC, P).T
    bv_ = bv.astype(f32).reshape(1, D)
    bob_ = bo.astype(MM_NP).reshape(1, D)
    in_maps = []
    per_batch = {}
    for b in range(B):
        idx = np.flatnonzero(np.asarray(mask[b]) != 0)
        n = len(idx)
        xk = np.zeros((kvcap, D), f32)
        xv = np.zeros((kvcap, D), f32)
        xk[:n] = np.asarray(key[b], f32)[idx]
        xv[:n] = np.asarray(value[b], f32)[idx]
        mbias = np.full(kvcap, MASK_BIAS, f32)
        mbias[:n] = 0.0
        cst = np.concatenate([bq_, bk_, mbias.reshape(kvc, P).T], axis=1)
        per_batch[b] = (
            np.ascontiguousarray(xk.T).astype(MM_NP),
            np.ascontiguousarray(xv.T).astype(MM_NP),
            np.ascontiguousarray(cst),
        )
    for c in range(N_CORES):
        b, half = c // 2, c % 2
        sl = slice(half * LQ, (half + 1) * LQ)
        xqT = np.ascontiguousarray(np.asarray(query[b], f32)[sl].T).astype(MM_NP)
        xkT_, xvT_, cst_ = per_batch[b]
        in_maps.append({
            "xqT": xqT, "xkT": xkT_, "xvT": xvT_,
            "wq": wq_, "wk": wk_, "wv": wv_, "wo": wo_,
            "cst": cst_, "bv": bv_, "bob": bob_,
        })
    return in_maps


def kernel(query, key, value, mask, Wq, bq, Wk, bk, Wv, bv, Wo, bo):
    kvc = _plan(mask)
    if kvc not in _cache:
        _cache[kvc] = _build(kvc)
    _cache["nc"] = _cache[kvc]
    nc = _cache[kvc]
    in_maps = _host_inputs(query, key, value, mask,
                           Wq, bq, Wk, bk, Wv, bv, Wo, bo, kvc=kvc)
    res = run_bass_kernel_spmd(nc, in_maps, list(range(N_CORES))).results
    out = np.empty((B, L, D), np.float32)
    for c in range(N_CORES):
        b, half = c // 2, c % 2
        out[b, half * LQ:(half + 1) * LQ, :] = res[c]["out"].astype(np.float32)
    return out
